# revision 4
# baseline (speedup 1.0000x reference)
"""Trainium2 Bass kernel for the 2-layer LIF spiking network (nn_Net_1460288881142).

Model (per step t, snnTorch Leaky reset-by-subtraction, BETA=0.9, TH=1):
    cur1 = x_t @ W1.T + b1                       (B, 8)
    mem1 = BETA*mem1 + cur1 - (mem1_prev > 1)    (reset uses PREVIOUS mem's spike)
    spk1 = (mem1 > 1)
    cur2 = spk1 @ W2.T + b2                      (B, 2)
    mem2 = BETA*mem2 + cur2 - (mem2_prev > 1)
    spk2 = (mem2 > 1)
Outputs: (spk2_rec, mem2_rec), each (S=8192, B=128, 2) float32.

Mapping to 8 NeuronCores:
 - Data-parallel over batch: 16 per core; tiny weights replicated; outputs
   concatenated on host. Lanes (b, h) = 128 on SBUF partitions.
 - Phase A1 (memory-bound part): stream x (48MB/core); cur1 via PE with the
   x-chunk as the STATIONARY operand, giving PSUM tiles [128 tau, (b,h)];
   ACT copy to SBUF; PE-transpose to [(b,h), tau]; ACT scatters them (bias
   folded in) into an interleaved layout col = j*NSUB + k.
 - Phase B1: the sequential scan, parallelized over NSUB=32 time sub-blocks,
   each preceded by W=256 warmup steps (leaky dynamics contract at 0.9^t, so
   a sub-block started from state 0 merges bit-exactly onto the true
   trajectory before its main region; validated offline on the actual
   inputs: zero spike flips end-to-end). State is the negated membrane
   w = -mem: two DVE scalar_tensor_tensor per wavefront step advance all 32
   sub-blocks at once:  t = (w*BETA) - c ;  w' = (w < -1) + t
   (this op order is bit-identical to the reference's (B*m + c) - s).
 - Phase A2: spk1 recovered chunkwise from the layer-1 state record; cur2
   via ONE PE matmul with a block-diagonal W2 stationary (K = 16*8 = 128),
   which lands directly as [(b,d), tau] - no transposes.
 - Phase B2: same wavefront scan for layer 2 (32 lanes).
"""
import os
import sys
import types

import numpy as np

# ---------------- constants (hardcoded problem geometry) ----------------
S = 8192
B = 128
H = 8
D_IN = 92
D_OUT = 2
N_CORES = 8
B_SH = B // N_CORES          # 16 batches per core
L1 = B_SH * H                # 128 layer-1 lanes per core
L2 = B_SH * D_OUT            # 32 layer-2 lanes per core
BETA = 0.9

NSUB = 32                    # time sub-blocks (both layers)
T_SUB = S // NSUB            # 256
W = 256                      # warmup steps (= T_SUB: each chunk's warmup
                             # copy duplicates the previous main chunk 1:1)
ST = W + T_SUB               # 512 wavefront steps
TCH = 128                    # tau-chunk for phase A1 (PE transpose tile)
NTCH = S // TCH              # 64
C2CH = 256                   # tau-chunk for phase A2
NC2 = S // C2CH              # 32

_cache = {}


def _install_compat():
    """walrus here rejects >1 sync wait per TPB instruction (0 on Drain);
    hoist excess waits onto same-engine NoOps. Also register the NTFF
    profiling hook trn_boot could not install (antenv.axon_hooks missing)."""
    import concourse.mybir as mybir

    def split_sync_waits(nc):
        engines_ok = {
            mybir.EngineType.PE, mybir.EngineType.Activation,
            mybir.EngineType.DVE, mybir.EngineType.Pool, mybir.EngineType.SP,
        }
        ctr = [0]

        def mk_nop(engine, wait):
            ctr[0] += 1
            return mybir.InstNoOp(
                name=f"waitnop-{ctr[0]}", engine=engine,
                sync_info=mybir.SyncInfo(on_wait=[wait], on_update=[]))

        for f in nc.m.functions:
            for bb in f.blocks:
                dirty = False
                new_insts = []
                for inst in bb.instructions:
                    si = inst.sync_info
                    if si is not None and si.on_wait and inst.engine in engines_ok:
                        maxw = 0 if str(inst.opcode) == "Drain" else 1
                        waits = list(si.on_wait)
                        if len(waits) > maxw:
                            keep = waits[len(waits) - maxw:] if maxw else []
                            for w_ in (waits[: len(waits) - maxw] if maxw else waits):
                                new_insts.append(mk_nop(inst.engine, w_))
                            si.on_wait = keep
                            dirty = True
                    new_insts.append(inst)
                if dirty:
                    bb.instructions = new_insts
        return nc

    def install_ntff_hook():
        try:
            import antenv
            hookmod = types.ModuleType("antenv.axon_hooks")
            hookmod._hook = None

            def set_axon_ntff_profile_hook(h):
                hookmod._hook = h

            def get_axon_ntff_profile_hook():
                return hookmod._hook

            hookmod.set_axon_ntff_profile_hook = set_axon_ntff_profile_hook
            hookmod.get_axon_ntff_profile_hook = get_axon_ntff_profile_hook
            sys.modules.setdefault("antenv.axon_hooks", hookmod)
            antenv.axon_hooks = sys.modules["antenv.axon_hooks"]
            from trn_agent_boot.trn_boot import _ntff_profile_via_ctypes
            sys.modules["antenv.axon_hooks"].set_axon_ntff_profile_hook(
                _ntff_profile_via_ctypes("/opt/axon/libaxon_pjrt.so"))
        except Exception:
            pass

    install_ntff_hook()
    return split_sync_waits


def _scol(buf, col0, step, count):
    """Strided free-axis AP: cols col0, col0+step, ... (count of them)."""
    return buf[:, col0:col0 + (count - 1) * step + 1:step]


def _build():
    """Build the per-core Bass module (SPMD: same program on all 8 cores)."""
    if "nc" in _cache:
        return _cache["nc"]
    import concourse.bass as bass
    import concourse.mybir as mybir
    from concourse import masks
    from concourse.tile import TileContext

    split_sync_waits = _install_compat()

    F32 = mybir.dt.float32
    A = mybir.AluOpType
    AF = mybir.ActivationFunctionType

    nc = bass.Bass()
    xT = nc.declare_dram_parameter("xT", [D_IN, B_SH, S], F32, isOutput=False)
    w1t_d = nc.declare_dram_parameter("W1T", [D_IN, H], F32, isOutput=False)
    b1c_d = nc.declare_dram_parameter("b1c", [L1, 1], F32, isOutput=False)
    w2b_d = nc.declare_dram_parameter("W2blk", [L1, L2], F32, isOutput=False)
    b2c_d = nc.declare_dram_parameter("b2c", [L2, 1], F32, isOutput=False)
    mem2_d = nc.declare_dram_parameter("mem2", [L2, S], F32, isOutput=True)
    spk2_d = nc.declare_dram_parameter("spk2", [L2, S], F32, isOutput=True)

    with TileContext(nc) as tc:
        with (
            tc.tile_pool(name="pers", bufs=1) as pers,
            tc.tile_pool(name="xin", bufs=3) as xin,
            tc.tile_pool(name="zt", bufs=3) as ztp,
            tc.tile_pool(name="spkc", bufs=3) as spkc,
            tc.tile_pool(name="psz", bufs=3, space="PSUM") as psz,
            tc.tile_pool(name="pst", bufs=3, space="PSUM") as pst,
            tc.tile_pool(name="psc", bufs=2, space="PSUM") as psc,
        ):
            w1t = pers.tile([D_IN, H], F32, tag="w1t")
            b1c = pers.tile([L1, 1], F32, tag="b1c")
            w2b = pers.tile([L1, L2], F32, tag="w2b")
            b2c = pers.tile([L2, 1], F32, tag="b2c")
            ident = pers.tile([128, 128], F32, tag="ident")
            nc.sync.dma_start(out=w1t[:], in_=w1t_d[:])
            nc.sync.dma_start(out=b1c[:], in_=b1c_d[:])
            nc.sync.dma_start(out=w2b[:], in_=w2b_d[:])
            nc.sync.dma_start(out=b2c[:], in_=b2c_d[:])
            masks.make_identity(nc, ident[:])

            c1i = pers.tile([L1, ST * NSUB], F32, tag="big1")     # 8MB
            wrec1 = pers.tile([L1, ST * NSUB], F32, tag="big2")   # 8MB
            dscr = pers.tile([L1, NSUB], F32, tag="dscr")
            winit = pers.tile([L1, NSUB], F32, tag="winit")
            nc.gpsimd.memset(c1i[:], 0.0)
            nc.vector.memset(winit[:], 0.0)

            # ---------------- Phase A1: cur1 -> c1i (interleaved) -----------
            for i in range(NTCH):
                tau0 = i * TCH
                k, r0 = tau0 // T_SUB, tau0 % T_SUB
                xck = xin.tile([D_IN, B_SH * TCH], F32, tag="xck")
                nc.sync.dma_start(out=xck[:], in_=xT[:, :, tau0:tau0 + TCH])
                pz = psz.tile([128, 128], F32, tag="pz")
                for b in range(B_SH):
                    nc.tensor.matmul(
                        pz[:, b * H:(b + 1) * H], xck[:, b * TCH:(b + 1) * TCH],
                        w1t[:], start=True, stop=True)
                zt = ztp.tile([128, 128], F32, tag="zt")
                nc.scalar.activation(out=zt[:], in_=pz[:], func=AF.Copy)
                ptr = pst.tile([128, 128], F32, tag="ptr")
                nc.tensor.transpose(ptr[:], zt[:], ident[:])
                # main region of sub-block k: col = (r + W)*NSUB + k
                nc.scalar.activation(
                    out=_scol(c1i, (r0 + W) * NSUB + k, NSUB, TCH),
                    in_=ptr[:], func=AF.Identity, bias=b1c[:], scale=1.0)
                # warmup region of sub-block k+1: col = r*NSUB + (k+1)
                if k + 1 < NSUB:
                    nc.scalar.activation(
                        out=_scol(c1i, r0 * NSUB + (k + 1), NSUB, TCH),
                        in_=ptr[:], func=AF.Identity, bias=b1c[:], scale=1.0)

            # ---------------- Phase B1: layer-1 wavefront scan --------------
            for j in range(ST):
                w_prev = winit[:] if j == 0 else wrec1[:, (j - 1) * NSUB:j * NSUB]
                # t = (w*BETA) - c
                nc.vector.scalar_tensor_tensor(
                    out=dscr[:], in0=w_prev, scalar=BETA,
                    in1=c1i[:, j * NSUB:(j + 1) * NSUB],
                    op0=A.mult, op1=A.subtract)
                # w' = (w < -1) + t
                nc.vector.scalar_tensor_tensor(
                    out=wrec1[:, j * NSUB:(j + 1) * NSUB], in0=w_prev,
                    scalar=-1.0, in1=dscr[:], op0=A.is_lt, op1=A.add)

            # ---------------- Phase A2: spk1 -> cur2 -> c2i -----------------
            c2i = pers.tile([L2, ST * NSUB], F32, tag="big1")     # reuses c1i
            wrec2 = pers.tile([L2, ST * NSUB], F32, tag="big2")   # reuses wrec1
            dscr2 = pers.tile([L2, NSUB], F32, tag="dscr2")
            nc.gpsimd.memset(c2i[:], 0.0)
            for i in range(NC2):
                tau0 = i * C2CH
                k, r0 = tau0 // T_SUB, tau0 % T_SUB
                spk = spkc.tile([L1, C2CH], F32, tag="spk")
                nc.vector.tensor_scalar(
                    out=spk[:],
                    in0=_scol(wrec1, (r0 + W) * NSUB + k, NSUB, C2CH),
                    scalar1=-1.0, scalar2=None, op0=A.is_lt)
                pc = psc.tile([L2, C2CH], F32, tag="pc")
                nc.tensor.matmul(pc[:], w2b[:], spk[:], start=True, stop=True)
                nc.scalar.activation(
                    out=_scol(c2i, (r0 + W) * NSUB + k, NSUB, C2CH),
                    in_=pc[:], func=AF.Identity, bias=b2c[:], scale=1.0)
                if k + 1 < NSUB:
                    nc.scalar.activation(
                        out=_scol(c2i, r0 * NSUB + (k + 1), NSUB, C2CH),
                        in_=pc[:], func=AF.Identity, bias=b2c[:], scale=1.0)

            # ---------------- Phase B2: layer-2 wavefront scan --------------
            for j in range(ST):
                w_prev = winit[:L2, :] if j == 0 else wrec2[:, (j - 1) * NSUB:j * NSUB]
                nc.vector.scalar_tensor_tensor(
                    out=dscr2[:], in0=w_prev, scalar=BETA,
                    in1=c2i[:, j * NSUB:(j + 1) * NSUB],
                    op0=A.mult, op1=A.subtract)
                nc.vector.scalar_tensor_tensor(
                    out=wrec2[:, j * NSUB:(j + 1) * NSUB], in0=w_prev,
                    scalar=-1.0, in1=dscr2[:], op0=A.is_lt, op1=A.add)

            # ---------------- Phase C: extract + store outputs --------------
            out2 = pers.tile([L2, 2 * S], F32, tag="big1")  # reuses c1i slot
            mem2_sb = out2[:, 0:S]
            spk2_sb = out2[:, S:2 * S]
            for k in range(NSUB):
                src = _scol(wrec2, W * NSUB + k, NSUB, T_SUB)
                # mem2 = -w2
                nc.scalar.activation(
                    out=mem2_sb[:, k * T_SUB:(k + 1) * T_SUB], in_=src,
                    func=AF.Copy, bias=0.0, scale=-1.0)
                # spk2 = (w2 < -1)
                nc.vector.tensor_scalar(
                    out=spk2_sb[:, k * T_SUB:(k + 1) * T_SUB], in0=src,
                    scalar1=-1.0, scalar2=None, op0=A.is_lt)
            nc.sync.dma_start(out=mem2_d[:], in_=mem2_sb)
            nc.sync.dma_start(out=spk2_d[:], in_=spk2_sb)

    split_sync_waits(nc)
    _cache["nc"] = nc
    return nc


def kernel(x, W1, b1, W2, b2):
    """Full inputs in, full outputs out. Shards batch over 8 NeuronCores."""
    sys.path.insert(0, "/opt/trn_rl_repo")
    from concourse.bass_utils import run_bass_kernel_spmd

    x = np.ascontiguousarray(np.asarray(x, dtype=np.float32))
    W1 = np.asarray(W1, dtype=np.float32)
    b1 = np.asarray(b1, dtype=np.float32)
    W2 = np.asarray(W2, dtype=np.float32)
    b2 = np.asarray(b2, dtype=np.float32)

    nc = _build()

    w1t_h = np.ascontiguousarray(W1.T)                        # (92, 8)
    b1c_h = np.ascontiguousarray(np.tile(b1, B_SH)[:, None])  # (128, 1)
    # block-diagonal W2: [(b',h), (b,d)] = W2[d, h] iff b' == b
    w2b_h = np.zeros((L1, L2), np.float32)
    for b in range(B_SH):
        w2b_h[b * H:(b + 1) * H, b * D_OUT:(b + 1) * D_OUT] = W2.T
    b2c_h = np.ascontiguousarray(np.tile(b2, B_SH)[:, None])  # (32, 1)

    in_maps = []
    for c in range(N_CORES):
        xc = x[c * B_SH:(c + 1) * B_SH]                       # (16, 4, 8192, 23)
        xTc = np.ascontiguousarray(
            xc.transpose(1, 3, 0, 2).reshape(D_IN, B_SH, S))  # (92, 16, 8192)
        in_maps.append({"xT": xTc, "W1T": w1t_h, "b1c": b1c_h,
                        "W2blk": w2b_h, "b2c": b2c_h})

    trace = bool(os.environ.get("TRN_KERNEL_TRACE"))
    res = run_bass_kernel_spmd(nc, in_maps, list(range(N_CORES)), trace=trace)
    if trace:
        _cache["last_exec_time_ns"] = res.exec_time_ns
        _cache["last_insts"] = res.instructions_and_trace

    spk = np.empty((S, B, D_OUT), np.float32)
    mem = np.empty((S, B, D_OUT), np.float32)
    for c in range(N_CORES):
        r = res.results[c]
        mem[:, c * B_SH:(c + 1) * B_SH, :] = (
            r["mem2"].reshape(B_SH, D_OUT, S).transpose(2, 0, 1))
        spk[:, c * B_SH:(c + 1) * B_SH, :] = (
            r["spk2"].reshape(B_SH, D_OUT, S).transpose(2, 0, 1))
    return spk, mem
